# revision 53
# baseline (speedup 1.0000x reference)
"""Trainium2 Bass kernel for nn_EnsembleModel (histogram_binning).

Math:
  hist[p,q]  = sum_{b,i,j} [adds[b,i]==p] * a_arc[b,i,j] * [adds[b,j]==q]
  score      = sigmoid(hist)                                  # [50,50]
  out[b,i,j] = s_arc[b,i,j] + ALPHA * score[pos[b,i], pos[b,j]]

Both the histogram and the gather-broadcast are expressed as TensorEngine
matmuls against one-hot matrices (U = onehot(adds), VT = onehot(pos).T)
computed on the host in partition-major layout (single dense DMA each):

  phase 1 (per batch):  P[p,jblk] = sum_i U[i,p] A[i,j]   (lhsT=U, rhs=A, N=512)
                        PT chunks = PE-transpose of P
                        hist     += PT.T @ U              (lhsT=PT, rhs=U)
  AllReduce(hist) over 8 cores, S' = ALPHA * sigmoid(hist)
  phase 2 (per batch):  GT[q,i] = sum_p S'[p,q] VT[p,i]   (lhsT=S', rhs=VT)
                        out     = s_arc + GT.T @ VT       (lhsT=GT slice, rhs=VT)

Everything rides bf16 (one-hot operands are exact in bf16; a_arc rounded on
host halves its HBM traffic; measured end-to-end L2 rel err ~4e-3).

Schedule (measured ~200us; DMA ~310 GB/s/core over 2 HWDGE rings):
  - a/s/out relayouted on host so every [128, 2, SL] tile is one 4 KiB
    contiguous run per partition (halves DMA descriptor count)
  - a-loads own both rings first -> local hist posts ~80us
  - a tiny AllReduce posted at t~0 warms the CC ring and absorbs cross-core
    launch skew; the real AllReduce's input bounce rides the then-empty sync
    ring so it triggers immediately after the last hist matmul
  - s-loads stream behind the a-loads / cc bounce; post-AR, PE (gather
    matmuls), DVE+Pool (adds, 3:1) and both DMA rings all run ~saturated
"""

import numpy as np
import ml_dtypes

ALPHA = 0.3
NP = 50          # n_pos
SL = 1024        # sequence length
BZ = 64          # global batch
NCORES = 8
B = BZ // NCORES  # local batch per core
NCH = SL // 128   # 128-row chunks per matrix
NBLK = SL // 512  # 512-col blocks per matrix
_CACHE = {}


def _build_nc():
    import concourse.bacc as bacc
    import concourse.mybir as mybir
    import concourse.tile as tile
    from concourse.tile import add_dep_helper

    f32 = mybir.dt.float32
    bf16 = mybir.dt.bfloat16
    nc = bacc.Bacc(
        "TRN2", target_bir_lowering=False, debug=False, num_devices=NCORES
    )

    # a/s/out are relayouted on the host so each [128, 2, SL] tile lands as
    # one 4 KiB-contiguous run per partition (halves DMA descriptor count):
    # x2[b, z, p, t, j] = x[b, 256 z + 128 t + p, j]
    NZ = NCH // 2
    a_d = nc.dram_tensor("a", [B, NZ, 128, 2, SL], bf16, kind="ExternalInput")
    s_d = nc.dram_tensor("s", [B, NZ, 128, 2, SL], bf16, kind="ExternalInput")
    u_d = nc.dram_tensor("u", [128, B, NCH, NP], bf16, kind="ExternalInput")
    vt_d = nc.dram_tensor("vt", [NP, B, SL], bf16, kind="ExternalInput")
    eye_d = nc.dram_tensor("eye", [NP, NP], bf16, kind="ExternalInput")
    out_d = nc.dram_tensor("out", [B, NZ, 128, 2, SL], bf16, kind="ExternalOutput")

    with tile.TileContext(nc) as tc:
        with (
            tc.tile_pool(name="const", bufs=1) as const_pool,
            tc.tile_pool(name="apool", bufs=8) as a_pool,
            tc.tile_pool(name="spool", bufs=22) as s_pool,
            tc.tile_pool(name="opool", bufs=11) as o_pool,
            tc.tile_pool(name="ppool", bufs=2) as p_pool,
            tc.tile_pool(name="ptsb", bufs=4) as pt_pool,
            tc.tile_pool(name="gtsb", bufs=2) as gt_pool,
            tc.tile_pool(name="ttsb", bufs=3) as tt_pool,
            tc.tile_pool(name="small", bufs=1) as small_pool,
            tc.tile_pool(name="dram", bufs=1, space="DRAM") as dram_pool,
        ):
            # Persistent operands — partition-major, one dense DMA each.
            u_sb = const_pool.tile([128, B, NCH, NP], bf16)
            eye_sb = const_pool.tile([NP, NP], bf16)
            vt_sb = const_pool.tile([NP, B, SL], bf16)
            nc.sync.dma_start(eye_sb[:], eye_d[:])
            nc.sync.dma_start(u_sb[:], u_d[:])
            nc.scalar.dma_start(vt_sb[:], vt_d[:])

            # Warm the sigmoid activation table off the critical path.
            warm = const_pool.tile([1, 1], f32)
            nc.vector.memset(warm[:], 0.0)
            nc.scalar.activation(
                warm[:], warm[:], mybir.ActivationFunctionType.Sigmoid
            )

            def run_cc(cc_in, cc_out):
                return nc.gpsimd.collective_compute(
                    "AllReduce",
                    mybir.AluOpType.add,
                    replica_groups=[list(range(NCORES))],
                    ins=[cc_in.opt()],
                    outs=[cc_out.opt()],
                )

            # Dummy collective posted immediately: absorbs the ~11us CC ring
            # startup latency plus initial cross-core launch skew, so the
            # real AllReduce sees a hot ring and synced peers. Tiny payload
            # so its mesh phases clear the CC ring quickly.
            w_sb = small_pool.tile([8, 8], f32, tag="wsb")
            nc.vector.memset(w_sb[:], 0.0)
            w_in = dram_pool.tile([8, 8], f32, tag="win")
            w_out = dram_pool.tile([8, 8], f32, tag="wout")
            nc.gpsimd.dma_start(w_in[:], w_sb[:])
            run_cc(w_in, w_out)

            last_a_load = [None]
            cc_gate = [None]
            # DRAM bounce tiles for the AllReduce. Bounce DMAs stay on gpsimd
            # (SWDGE) so they never queue behind bulk HWDGE traffic.
            # bf16 payload: halves the bytes each mesh hop moves (the hops crawl
            # under HBM contention); one rounding of the accumulated hist is
            # numerically negligible vs the bf16 inputs.
            cc_in = dram_pool.tile([NP, NP], bf16, tag="ccin")
            cc_out = dram_pool.tile([NP, NP], bf16, tag="ccout")

            # ---- Phase 1: local histogram (a-loads own both DMA rings) ----
            with (
                tc.tile_pool(name="histps", bufs=2, space="PSUM") as hist_pool,
                tc.tile_pool(name="pps", bufs=2, space="PSUM") as pps_pool,
                tc.tile_pool(name="tpps", bufs=2, space="PSUM") as tpps_pool,
            ):
                hist_ps = hist_pool.tile([NP, NP], f32, tag="hist")
                for b in range(B):
                    a_tiles = []
                    for z in range(NZ):
                        at = a_pool.tile([128, 2, SL], bf16, tag="a")
                        eng = nc.sync if (z % 2 == 0) else nc.scalar
                        ld = eng.dma_start(at[:], a_d[b, z])
                        last_a_load[0] = ld.ins
                        a_tiles.append(at)
                    # P[p, j] = sum_i U[i,p] A[i,j], N=512 moving A.
                    # ic-outer: each stationary U chunk streams both 512-col
                    # blocks back to back (longer PE streaks, fewer stalls).
                    p_sb = p_pool.tile([NP, SL], bf16, tag="p")
                    p_ps0 = pps_pool.tile([NP, 512], f32, tag="pp")
                    p_ps1 = pps_pool.tile([NP, 512], f32, tag="pp")
                    p_ps = [p_ps0, p_ps1]
                    for ic in range(NCH):
                        for jb in range(NBLK):
                            nc.tensor.matmul(
                                p_ps[jb][:],
                                u_sb[:, b, ic, :],
                                a_tiles[ic // 2][
                                    :, ic % 2, jb * 512:(jb + 1) * 512
                                ],
                                start=(ic == 0),
                                stop=(ic == NCH - 1),
                            )
                    for jb in range(NBLK):
                        nc.vector.tensor_copy(
                            p_sb[:, jb * 512:(jb + 1) * 512], p_ps[jb][:]
                        )
                    # hist += PT.T @ U per 128-chunk of j.
                    for jc in range(NCH):
                        tp_ps = tpps_pool.tile([128, NP], bf16, tag="tp")
                        nc.tensor.transpose(
                            tp_ps[:], p_sb[:, jc * 128:(jc + 1) * 128], eye_sb[:]
                        )
                        pts = pt_pool.tile([128, NP], bf16, tag="pts")
                        nc.scalar.copy(pts[:], tp_ps[:])
                        nc.tensor.matmul(
                            hist_ps[:],
                            pts[:],
                            u_sb[:, b, jc, :],
                            start=(b == 0 and jc == 0),
                            stop=(b == B - 1 and jc == NCH - 1),
                        )
                h0 = small_pool.tile([NP, NP], bf16, tag="h0")
                nc.vector.tensor_copy(h0[:], hist_ps[:])
                # The bounce write rides the sync HWDGE ring, which is empty
                # at this point (a-loads done, s-loads gated behind this very
                # DMA) — so the collective triggers with minimal latency.
                cc_in_ld = nc.sync.dma_start(cc_in[:], h0[:])
                cc_gate[0] = cc_in_ld.ins
                run_cc(cc_in, cc_out)

            hsum = small_pool.tile([NP, NP], bf16, tag="hsum")
            nc.gpsimd.dma_start(hsum[:], cc_out[:])

            # ---- sigmoid + alpha ----
            sg = small_pool.tile([NP, NP], f32, tag="sg")
            nc.scalar.activation(
                sg[:], hsum[:], mybir.ActivationFunctionType.Sigmoid
            )
            sc = small_pool.tile([NP, NP], bf16, tag="sc")
            nc.vector.tensor_scalar_mul(sc[:], sg[:], ALPHA)

            # ---- Phase 2: broadcast-back + add (DMA-bound; DVE+Pool adds) ----
            with (
                tc.tile_pool(name="gtps", bufs=2, space="PSUM") as gtps_pool,
                tc.tile_pool(name="ops", bufs=3, space="PSUM") as ops_pool,
            ):
                for b in range(B):
                    gt_sb = gt_pool.tile([NP, SL], bf16, tag="gt")
                    for ib in range(NBLK):
                        gt_ps = gtps_pool.tile([NP, 512], f32, tag="gtp")
                        nc.tensor.matmul(
                            gt_ps[:],
                            sc[:],
                            vt_sb[:, b, ib * 512:(ib + 1) * 512],
                            start=True,
                            stop=True,
                        )
                        nc.scalar.copy(
                            gt_sb[:, ib * 512:(ib + 1) * 512], gt_ps[:]
                        )
                    for z in range(NZ):
                        kz = b * NZ + z
                        st = s_pool.tile([128, 2, SL], bf16, tag="s")
                        s_eng = nc.sync if (z % 2 == 0) else nc.scalar
                        sld = s_eng.dma_start(st[:], s_d[b, z])
                        # First few s transfers fill the DMA gap between the
                        # last a-load and the histogram tail; the rest stay
                        # behind the collective's input bounce so it never
                        # queues behind bulk traffic.
                        add_dep_helper(
                            sld.ins,
                            last_a_load[0] if kz < 12 else cc_gate[0],
                            reason="s-loads after a-loads / cc bounce",
                        )
                        ot = o_pool.tile([128, 2, SL], bf16, tag="o")
                        for t in range(2):
                            c = 2 * z + t
                            k = b * NCH + c
                            o_ps = ops_pool.tile([128, SL], f32, tag="op")
                            for jb in range(NBLK):
                                nc.tensor.matmul(
                                    o_ps[:, jb * 512:(jb + 1) * 512],
                                    gt_sb[:, c * 128:(c + 1) * 128],
                                    vt_sb[:, b, jb * 512:(jb + 1) * 512],
                                    start=True,
                                    stop=True,
                                )
                            if k % 4 == 1:
                                # Pool can't read PSUM: ACT stages the matmul
                                # result to SBUF bf16, Pool adds there.
                                tt = tt_pool.tile([128, SL], bf16, tag="tt")
                                nc.scalar.copy(tt[:], o_ps[:])
                                nc.gpsimd.tensor_add(
                                    ot[:, t, :], st[:, t, :], tt[:]
                                )
                            else:
                                nc.vector.tensor_add(
                                    ot[:, t, :], st[:, t, :], o_ps[:]
                                )
                        out_eng = nc.sync if (z % 2 == 1) else nc.scalar
                        out_eng.dma_start(out_d[b, z], ot[:])

    nc.compile()
    return nc


def _get_nc():
    if "nc" not in _CACHE:
        _CACHE["nc"] = _build_nc()
    return _CACHE["nc"]


def kernel(a_arc, s_arc, adds, pos, n_pos, _trace=False, _return_perf=False):
    from concourse.bass_utils import run_bass_kernel_spmd

    assert int(n_pos) == NP
    a = np.asarray(a_arc, dtype=np.float32)
    s = np.asarray(s_arc, dtype=np.float32)
    adds = np.asarray(adds)
    pos = np.asarray(pos)

    rng = np.arange(NP)
    eye = np.eye(NP, dtype=ml_dtypes.bfloat16)

    def relayout(x):
        # [B, SL, SL] -> [B, NZ, 128, 2, SL]: x2[b,z,p,t,j] = x[b,256z+128t+p,j]
        return np.ascontiguousarray(
            x.reshape(B, NCH // 2, 2, 128, SL).swapaxes(2, 3)
        )

    in_maps = []
    for k in range(NCORES):
        sl = slice(k * B, (k + 1) * B)
        adds_sh = adds[sl]
        pos_sh = pos[sl]
        # u[p, b, c, q] = [adds[b, c*128+p] == q]  (partition-major)
        u2 = (
            adds_sh.reshape(B, NCH, 128).transpose(2, 0, 1)[..., None] == rng
        ).astype(ml_dtypes.bfloat16)
        # vt[p, b, i] = [pos[b, i] == p]
        vtb = (rng[:, None, None] == pos_sh[None, :, :]).astype(
            ml_dtypes.bfloat16
        )
        in_maps.append(
            {
                "a": relayout(a[sl].astype(ml_dtypes.bfloat16)),
                "s": relayout(s[sl].astype(ml_dtypes.bfloat16)),
                "u": np.ascontiguousarray(u2),
                "vt": np.ascontiguousarray(vtb),
                "eye": eye,
            }
        )

    nc = _get_nc()
    res = run_bass_kernel_spmd(
        nc, in_maps, core_ids=list(range(NCORES)), trace=_trace
    )
    # Undo the [B, NZ, 128, 2, SL] relayout per core shard.
    out = np.concatenate(
        [
            r["out"].swapaxes(2, 3).reshape(B, SL, SL)
            for r in res.results
        ],
        axis=0,
    ).astype(np.float32)
    if _return_perf:
        return out, res
    return out
